# revision 1
# baseline (speedup 1.0000x reference)
"""
DeepAttMISL segment-reduce kernel for Trainium2 (Bass/Tile), 8 NeuronCores.

Math (see reference):
  h        = relu(x @ W1.T + b1)                    x:[N,1024] -> h:[N,256]
  seg      = segment_sum(h, cluster_id, 8)          -> [8,256]
  h_clust  = seg / max(counts,1)
  h_path   = relu(h_clust @ Wf.T + bf)
  A        = softmax((tanh(h_path@Wa.T+ba) * sigmoid(h_path@Wb.T+bb)) @ Wc.T + bc)
  H        = A @ h_path                             -> [1,256]

Sharding: rows (instances) N=65536 are split across 8 cores (8192 each).
Each core computes h for its shard and the per-cluster partial sums
(one-hot matmul on the tensor engine); the partials are AllReduce'd
across the 8 cores, and every core redundantly computes the tiny
attention head; core 0's output is returned.

Precision: the big matmul runs in bf16 (inputs rounded once on host)
with fp32 PSUM accumulation; everything from the segment sums onward is
fp32. The softmax skips the max-subtraction: its logits are bounded
(|a*g| < 1, Wc row norm ~1, bc tiny), so exp() is safe in fp32.
"""

import sys

if "/opt/trn_rl_repo" not in sys.path:
    sys.path.insert(0, "/opt/trn_rl_repo")

import numpy as np
import ml_dtypes

import concourse.bass as bass
import concourse.tile as tile
from concourse import bacc, mybir
from concourse import bass_utils

N_CORES = 8
N_TOTAL = 65536
N_SHARD = N_TOTAL // N_CORES          # 8192 rows per core
DIN = 1024
DHID = 256
K_CL = 8                               # clusters
KC = DIN // 128                        # 8 contraction chunks
ROWT = N_SHARD // 128                  # 64 row-tiles of 128 rows
# superblock row sizes: small first blocks so the PE starts early
SB_SIZES = [768, 1280, 1536, 1536, 1536, 1536]
assert sum(SB_SIZES) == N_SHARD

BF16 = mybir.dt.bfloat16
F32 = mybir.dt.float32
AF = mybir.ActivationFunctionType
ALU = mybir.AluOpType

_CACHE = {}


def _build_nc():
    nc = bacc.Bacc("TRN2", target_bir_lowering=False, debug=False,
                   num_devices=N_CORES)

    # ---- per-core external inputs ----
    xT = nc.dram_tensor("xT", [DIN, N_SHARD], BF16, kind="ExternalInput")
    moh = nc.dram_tensor("moh", [128, ROWT, K_CL], BF16, kind="ExternalInput")
    w1t = nc.dram_tensor("w1t", [DIN, DHID], BF16, kind="ExternalInput")
    b1b = nc.dram_tensor("b1b", [128, DHID], F32, kind="ExternalInput")
    invc = nc.dram_tensor("invc", [K_CL, 1], F32, kind="ExternalInput")
    ident = nc.dram_tensor("ident", [K_CL, K_CL], F32, kind="ExternalInput")
    wft = nc.dram_tensor("wft", [DHID, DHID], F32, kind="ExternalInput")
    wat = nc.dram_tensor("wat", [DHID, DHID], F32, kind="ExternalInput")
    wbt = nc.dram_tensor("wbt", [DHID, DHID], F32, kind="ExternalInput")
    wcr = nc.dram_tensor("wcr", [DHID, 128], F32, kind="ExternalInput")
    bfc = nc.dram_tensor("bfc", [128, 2], F32, kind="ExternalInput")
    bac = nc.dram_tensor("bac", [128, 2], F32, kind="ExternalInput")
    bbc = nc.dram_tensor("bbc", [128, 2], F32, kind="ExternalInput")
    bcr = nc.dram_tensor("bcr", [128, 1], F32, kind="ExternalInput")

    out = nc.dram_tensor("out", [1, DHID], F32, kind="ExternalOutput")

    with tile.TileContext(nc) as tc:
        with tc.tile_pool(name="consts", bufs=1) as consts, \
             tc.tile_pool(name="xblk", bufs=3) as xblk, \
             tc.tile_pool(name="hpool", bufs=8) as hpool, \
             tc.tile_pool(name="hps", bufs=3, space="PSUM") as hps, \
             tc.tile_pool(name="segps", bufs=1, space="PSUM") as segps, \
             tc.tile_pool(name="headps", bufs=2, space="PSUM") as headps, \
             tc.tile_pool(name="small", bufs=1) as small, \
             tc.tile_pool(name="dram", bufs=1, space="DRAM") as dram:

            # ---- critical-path constants on the ACT ring, x data on the SP
            # ring — the two HWDGE rings load in parallel ----
            w1t_sb = consts.tile([128, KC, DHID], BF16)
            nc.scalar.dma_start(w1t_sb[:], w1t.ap().rearrange("(k p) f -> p k f", p=128))

            # first x superblock right behind it, before everything else
            xts_blocks = []
            row0 = 0
            for sb, sbr in enumerate(SB_SIZES):
                xts = xblk.tile([128, KC, sbr], BF16, tag="xts",
                                padded_shape=[128, KC, max(SB_SIZES)],
                                name=f"xts{sb}")
                xts_blocks.append(xts)
                if sb == 0:
                    for k in range(KC):
                        eng = nc.sync if k % 2 == 0 else nc.scalar
                        eng.dma_start(
                            xts[:, k, :],
                            xT.ap()[k * 128:(k + 1) * 128, row0:row0 + sbr])
                row0 += sbr

            b1b_sb = consts.tile([128, DHID], F32)
            nc.scalar.dma_start(b1b_sb[:], b1b.ap())
            m_sb = consts.tile([128, ROWT, K_CL], BF16)
            nc.scalar.dma_start(m_sb[:], moh.ap())

            # ---- non-critical constants on the ACT HWDGE ring ----
            invc_sb = consts.tile([K_CL, 1], F32)
            nc.scalar.dma_start(invc_sb[:], invc.ap())
            id_sb = consts.tile([K_CL, K_CL], F32)
            nc.scalar.dma_start(id_sb[:], ident.ap())
            wft_sb = consts.tile([128, 2, DHID], F32)
            nc.scalar.dma_start(wft_sb[:], wft.ap().rearrange("(i p) f -> p i f", p=128))
            wat_sb = consts.tile([128, 2, DHID], F32)
            nc.scalar.dma_start(wat_sb[:], wat.ap().rearrange("(i p) f -> p i f", p=128))
            wbt_sb = consts.tile([128, 2, DHID], F32)
            nc.scalar.dma_start(wbt_sb[:], wbt.ap().rearrange("(i p) f -> p i f", p=128))
            wcr_sb = consts.tile([128, 2, 128], F32)
            nc.scalar.dma_start(wcr_sb[:], wcr.ap().rearrange("(i p) f -> p i f", p=128))
            bfc_sb = consts.tile([128, 2], F32)
            nc.scalar.dma_start(bfc_sb[:], bfc.ap())
            bac_sb = consts.tile([128, 2], F32)
            nc.scalar.dma_start(bac_sb[:], bac.ap())
            bbc_sb = consts.tile([128, 2], F32)
            nc.scalar.dma_start(bbc_sb[:], bbc.ap())
            bcr_sb = consts.tile([128, 1], F32)
            nc.scalar.dma_start(bcr_sb[:], bcr.ap())

            # ---- segment-sum accumulator (lives across the whole loop) ----
            seg_ps = segps.tile([K_CL, DHID], F32)

            # ---- main loop ----
            pending_seg = []
            row0 = 0
            for sb, sbr in enumerate(SB_SIZES):
                tpb = sbr // 128
                xts = xts_blocks[sb]
                if sb > 0:
                    for k in range(KC):
                        eng = nc.sync if k % 2 == 0 else nc.scalar
                        eng.dma_start(
                            xts[:, k, :],
                            xT.ap()[k * 128:(k + 1) * 128, row0:row0 + sbr])
                for tl in range(tpb):
                    t = row0 // 128 + tl
                    hp = hps.tile([128, DHID], F32)
                    for k in range(KC):
                        nc.tensor.matmul(
                            hp[:],
                            xts[:, k, tl * 128:(tl + 1) * 128],
                            w1t_sb[:, k, :],
                            start=(k == 0), stop=(k == KC - 1),
                            skip_group_check=True)
                    nc.vector.tensor_add(hp[:], hp[:], b1b_sb[:])
                    h_sb = hpool.tile([128, DHID], BF16)
                    nc.scalar.activation(h_sb[:], hp[:], AF.Relu)
                    pending_seg.append((h_sb, t))
                    # flush seg matmuls in batches to keep the x-weight
                    # LDWEIGHTS pipeline unbroken for longer runs
                    if len(pending_seg) >= 5:
                        while len(pending_seg) > 1:
                            ph, pt = pending_seg.pop(0)
                            nc.tensor.matmul(
                                seg_ps[:], m_sb[:, pt, :], ph[:],
                                start=(pt == 0), stop=False,
                                skip_group_check=True)
                row0 += sbr
            while pending_seg:
                ph, pt = pending_seg.pop(0)
                nc.tensor.matmul(seg_ps[:], m_sb[:, pt, :], ph[:],
                                 start=(pt == 0), stop=(len(pending_seg) == 0),
                                 skip_group_check=True)

            # ---- AllReduce partial segment sums across the 8 cores ----
            seg_sb = small.tile([K_CL, DHID], F32)
            nc.vector.tensor_copy(seg_sb[:], seg_ps[:])
            ar_in = dram.tile([K_CL, DHID], F32)
            ar_out = dram.tile([K_CL, DHID], F32)
            nc.sync.dma_start(ar_in[:], seg_sb[:])
            nc.gpsimd.collective_compute(
                "AllReduce", ALU.add,
                replica_groups=[list(range(N_CORES))],
                ins=[ar_in[:].opt()], outs=[ar_out[:].opt()])
            ar_sb = small.tile([K_CL, DHID], F32)
            nc.sync.dma_start(ar_sb[:], ar_out[:])

            # ---- cluster means ----
            hc_sb = small.tile([K_CL, DHID], F32)
            nc.vector.tensor_scalar_mul(hc_sb[:], ar_sb[:], invc_sb[:, 0:1])

            # ---- transpose hc [8,256] -> hcT [256(2x128),8] via PE ----
            hcT = small.tile([128, 2, K_CL], F32)
            for j in range(2):
                tp = headps.tile([128, K_CL], F32, tag="head",
                                 padded_shape=[128, 128])
                nc.tensor.transpose(tp[:], hc_sb[:, j * 128:(j + 1) * 128], id_sb[:])
                nc.vector.tensor_copy(hcT[:, j, :], tp[:])

            # ---- attention head (fp32, transposed layout: [hid, k]) ----
            def head_mm(wt_sb, rhs, bias_sb, func, name):
                o = small.tile([128, 2, K_CL], F32, name=name)
                for j in range(2):
                    ps = headps.tile([128, K_CL], F32, tag="head",
                                     padded_shape=[128, 128])
                    for i in range(2):
                        nc.tensor.matmul(ps[:], wt_sb[:, i, j * 128:(j + 1) * 128],
                                         rhs[:, i, :],
                                         start=(i == 0), stop=(i == 1))
                    nc.scalar.activation(o[:, j, :], ps[:], func,
                                         bias=bias_sb[:, j:j + 1])
                return o

            hpT = head_mm(wft_sb, hcT, bfc_sb, AF.Relu, "hpT")
            aT = head_mm(wat_sb, hpT, bac_sb, AF.Tanh, "aT")
            gT = head_mm(wbt_sb, hpT, bbc_sb, AF.Sigmoid, "gT")
            agT = small.tile([128, 2, K_CL], F32)
            nc.vector.tensor_mul(agT[:], aT[:], gT[:])

            # logits replicated on all 128 partitions:
            # A_pre[p, k] = sum_hid Wc[0,hid]*ag[k,hid]  (wcr = Wc.T tiled 128x)
            a_ps = headps.tile([128, K_CL], F32, tag="head",
                               padded_shape=[128, 128])
            for j in range(2):
                nc.tensor.matmul(a_ps[:], wcr_sb[:, j, :], agT[:, j, :],
                                 start=(j == 0), stop=(j == 1))
            a_sb = small.tile([128, K_CL], F32)
            nc.vector.tensor_scalar_add(a_sb[:], a_ps[:], bcr_sb[:, 0:1])

            # softmax over the 8 clusters (bounded logits; skip max-shift)
            ea = small.tile([128, K_CL], F32)
            nc.scalar.activation(ea[:], a_sb[:], AF.Exp)
            ssum = small.tile([128, 1], F32)
            nc.vector.reduce_sum(ssum[:], ea[:], axis=mybir.AxisListType.X)
            rs = small.tile([128, 1], F32)
            nc.vector.reciprocal(rs[:], ssum[:])
            an = small.tile([128, K_CL], F32)
            nc.vector.tensor_scalar_mul(an[:], ea[:], rs[:, 0:1])

            # H[hid] = sum_k A[k] * h_path.T[hid, k]  (fused mul+reduce)
            h_out = small.tile([128, 2], F32)
            for j in range(2):
                tmp = small.tile([128, K_CL], F32, name=f"wtmp{j}")
                nc.vector.tensor_mul(tmp[:], hpT[:, j, :], an[:])
                nc.vector.reduce_sum(h_out[:, j:j + 1], tmp[:],
                                     axis=mybir.AxisListType.X)
                nc.sync.dma_start(out.ap()[0:1, j * 128:(j + 1) * 128],
                                  h_out[:, j:j + 1])

    nc.compile()
    return nc


def _prep_inputs(x_path, cluster_id, W1, b1, Wf, bf, Wa, ba, Wb, bb, Wc, bc):
    """Host-side sharding / marshalling. Returns in_maps for the 8 cores."""
    x = np.asarray(x_path, dtype=np.float32).reshape(N_TOTAL, DIN)
    cid = np.asarray(cluster_id).astype(np.int64).reshape(N_TOTAL)

    xb = x.astype(ml_dtypes.bfloat16)

    # one-hot cluster matrix, pre-tiled to [128, ROWT, K] per core
    oh = (cid[:, None] == np.arange(K_CL)[None, :]).astype(ml_dtypes.bfloat16)

    counts = np.bincount(cid, minlength=K_CL).astype(np.float32)
    invc = (1.0 / np.maximum(counts, 1.0)).reshape(K_CL, 1).astype(np.float32)

    W1 = np.asarray(W1, np.float32); b1 = np.asarray(b1, np.float32)
    Wf = np.asarray(Wf, np.float32); bf = np.asarray(bf, np.float32)
    Wa = np.asarray(Wa, np.float32); ba = np.asarray(ba, np.float32)
    Wb = np.asarray(Wb, np.float32); bb = np.asarray(bb, np.float32)
    Wc = np.asarray(Wc, np.float32); bc = np.asarray(bc, np.float32)

    const_map = {
        "w1t": np.ascontiguousarray(W1.T).astype(ml_dtypes.bfloat16),
        "b1b": np.ascontiguousarray(np.broadcast_to(b1, (128, DHID))),
        "invc": invc,
        "ident": np.eye(K_CL, dtype=np.float32),
        "wft": np.ascontiguousarray(Wf.T),
        "wat": np.ascontiguousarray(Wa.T),
        "wbt": np.ascontiguousarray(Wb.T),
        "wcr": np.ascontiguousarray(np.broadcast_to(Wc.T, (DHID, 128))),
        "bfc": np.ascontiguousarray(bf.reshape(2, 128).T),
        "bac": np.ascontiguousarray(ba.reshape(2, 128).T),
        "bbc": np.ascontiguousarray(bb.reshape(2, 128).T),
        "bcr": np.full((128, 1), float(bc.ravel()[0]), np.float32),
    }

    in_maps = []
    for c in range(N_CORES):
        lo, hi = c * N_SHARD, (c + 1) * N_SHARD
        xT_c = np.ascontiguousarray(xb[lo:hi].T)            # [1024, 8192] bf16
        moh_c = np.ascontiguousarray(
            oh[lo:hi].reshape(ROWT, 128, K_CL).transpose(1, 0, 2))
        in_maps.append({"xT": xT_c, "moh": moh_c, **const_map})
    return in_maps


def kernel(**inputs):
    if "nc" not in _CACHE:
        _CACHE["nc"] = _build_nc()
    nc = _CACHE["nc"]
    in_maps = _prep_inputs(**inputs)
    res = bass_utils.run_bass_kernel_spmd(
        nc, in_maps, core_ids=list(range(N_CORES)))
    return res.results[0]["out"].astype(np.float32)

